# revision 2
# baseline (speedup 1.0000x reference)
"""Trainium2 Bass kernel for nn_ASSANetEncoder (point-cloud set-abstraction encoder).

Reference computation per batch b (B=8, N=16384, P=4096, S=32, C_in=64, C=128):
    neigh[c,p,s] = support_features[c, idx[p,s]]
    rel[d,p,s]   = support_xyz[idx[p,s], d] - query_xyz[p, d]
    agg[c,p,s]   = neigh[c,p,s] * rel[d(c),p,s]      (d(c): 21/21/22 repeat-interleave)
    y1 = relu(scale1*(W1@agg) + shift1)              (inference BN folded to scale/shift)
    y2 = scale2*(W2@y1) + shift2
    out[o,p]     = max_s relu(y2)

Sharding: data-parallel over batch, one batch per NeuronCore (8 cores).

Per-core kernel strategy:
  * A per-batch fp16 table with rows [f*g_rep (64ch) | f (64ch)] lives in SBUF;
    rows are fetched with SBUF-source dma_gather(transpose=True), which lands
    gathered rows directly in [channel-on-partition, point-on-free] layout.
  * agg = f*(g_rep - q_rep) is decomposed as f*g_rep - f*q_rep:
      - the f*g_rep product is precomputed per support point (table top half)
      - f*q_rep is one fp16 tensor_tensor multiply on the gathered bottom half
      - the subtraction folds into the first matmul by stacking [W1'; -W1']
        as a K=128 lhsT.
  * BN scales fold into the conv weights; shifts ride the ReLU activations as
    per-partition bias. max over S commutes with the final (bias+relu).
  * Gathers are issued once per s value (4096 indices each) to amortize SWDGE
    descriptor generation; matmuls consume 512-column chunks (one PSUM bank).
"""

import os
import sys

sys.path.insert(0, "/opt/trn_rl_repo")

import numpy as np

B, N, NPOINT, NSAMPLE = 8, 16384, 4096, 32
C_IN, C_MID, C_OUT = 64, 128, 128
EPS = 1e-5
REPEATS = [21, 21, 22]

CHUNK = 512                    # matmul free dim / PSUM bank
NCHUNK = NPOINT // CHUNK       # 8 chunks per gather
GIDX = NPOINT                  # indices per dma_gather (one full s slice)

_compiled = None


def _build():
    import concourse.tile as tile
    from concourse import bacc, mybir

    f16 = mybir.dt.float16
    f32 = mybir.dt.float32
    i16 = mybir.dt.int16
    Alu = mybir.AluOpType
    Act = mybir.ActivationFunctionType

    nc = bacc.Bacc("TRN2", target_bir_lowering=False, debug=False,
                   enable_asserts=False, num_devices=8)

    table_d = nc.dram_tensor("table", [128, N], f16, kind="ExternalInput")
    idx_d = nc.dram_tensor("idx", [128, NPOINT * NSAMPLE // 16], i16,
                           kind="ExternalInput")
    qi_d = nc.dram_tensor("qi", [C_IN, NPOINT], f16, kind="ExternalInput")
    w_d = nc.dram_tensor("wstack", [128, 256], f16, kind="ExternalInput")
    c_d = nc.dram_tensor("consts", [128, 2], f32, kind="ExternalInput")
    out_d = nc.dram_tensor("out", [C_OUT, NPOINT], f32, kind="ExternalOutput")

    with tile.TileContext(nc) as tc:
        with (
            tc.tile_pool(name="const", bufs=1) as cpool,
            tc.tile_pool(name="g", bufs=3) as gpool,
            tc.tile_pool(name="y1r", bufs=4) as rpool,
            tc.tile_pool(name="ps1", bufs=2, space="PSUM") as ps1,
            tc.tile_pool(name="ps2", bufs=4, space="PSUM") as ps2,
        ):
            table = cpool.tile([128, N], f16, tag="table")
            nc.sync.dma_start(table[:], table_d.ap()[:])
            idx = cpool.tile([128, NPOINT * NSAMPLE // 16], i16, tag="idx")
            nc.sync.dma_start(idx[:], idx_d.ap()[:])
            qi = cpool.tile([128, NPOINT], f16, tag="qi")
            nc.sync.dma_start(qi[64:128, :], qi_d.ap()[:])
            w = cpool.tile([128, 256], f16, tag="w")
            nc.sync.dma_start(w[:], w_d.ap()[:])
            consts = cpool.tile([128, 2], f32, tag="consts")
            nc.sync.dma_start(consts[:], c_d.ap()[:])
            acc = cpool.tile([128, NPOINT], f32, tag="acc")
            outt = cpool.tile([128, NPOINT], f32, tag="outt")

            nreg = nc.gpsimd.to_reg(GIDX)

            for s in range(NSAMPLE):
                G = gpool.tile([128, GIDX], f16, tag="G")
                nc.gpsimd.dma_gather(
                    G[:].rearrange("p (a n) -> p a n", a=1),
                    table[:],
                    idx[:, s * (GIDX // 16):(s + 1) * (GIDX // 16)],
                    GIDX,
                    nreg,
                    128,
                    transpose=True,
                    # single_packet=True packs all descriptors into one DMA
                    # packet; beyond ~64 descriptors that wedges the device.
                    single_packet=False,
                    sbuf_tokens_per_rank=128,
                    sbuf_free_dim_per_rank=256,
                )
                # bottom half: f * q_rep  (in place)
                nc.vector.tensor_tensor(G[64:128, :], G[64:128, :],
                                        qi[64:128, :], Alu.mult)
                for c in range(NCHUNK):
                    cs = slice(c * CHUNK, (c + 1) * CHUNK)
                    y1 = ps1.tile([128, CHUNK], f32, tag="y1")
                    nc.tensor.matmul(y1[:], w[:, 0:128], G[:, cs],
                                     start=True, stop=True)
                    y1r = rpool.tile([128, CHUNK], f16, tag="y1r")
                    nc.scalar.activation(y1r[:], y1[:], Act.Relu,
                                         bias=consts[:, 0:1], scale=1.0)
                    y2 = ps2.tile([128, CHUNK], f32, tag="y2")
                    nc.tensor.matmul(y2[:], w[:, 128:256], y1r[:],
                                     start=True, stop=True)
                    if s == 0:
                        nc.scalar.activation(acc[:, cs], y2[:], Act.Copy)
                    else:
                        nc.vector.tensor_tensor(acc[:, cs], y2[:], acc[:, cs],
                                                Alu.max)

            nc.scalar.activation(outt[:], acc[:], Act.Relu,
                                 bias=consts[:, 1:2], scale=1.0)
            nc.sync.dma_start(out_d.ap()[:], outt[:])

    nc.compile()
    return nc


def _get_compiled():
    global _compiled
    if _compiled is None:
        _compiled = _build()
    return _compiled


def _prep_core_inputs(b, query_xyz, support_xyz, support_features, neighbor_idx,
                      wstack, consts):
    f = np.asarray(support_features[b], np.float32)            # [64, N]
    grep = np.repeat(np.asarray(support_xyz[b], np.float32).T,
                     REPEATS, axis=0)                          # [64, N]
    rows = np.concatenate([(f * grep).T, f.T], axis=1).astype(np.float16)
    # SBUF table layout: partition = row % 128, rank (free 256B slot) = row // 128
    table = np.ascontiguousarray(
        rows.reshape(N // 128, 128, 128).transpose(1, 0, 2).reshape(128, N))

    stream = np.asarray(neighbor_idx[b], np.int64).T.reshape(-1)  # [S*P], p fastest
    wrapped = stream.astype(np.int16).reshape(-1, 16).T           # [16, S*P/16]
    idx = np.ascontiguousarray(np.tile(wrapped, (8, 1)))          # [128, S*P/16]

    qi = np.ascontiguousarray(
        np.repeat(np.asarray(query_xyz[b], np.float32).T, REPEATS, axis=0)
    ).astype(np.float16)                                          # [64, P]

    return {"table": table, "idx": idx, "qi": qi,
            "wstack": wstack, "consts": consts}


def kernel(query_xyz, support_xyz, support_features, neighbor_idx,
           W1, g1, b1, m1, v1, W2, g2, b2, m2, v2):
    from concourse.bass_utils import run_bass_kernel_spmd

    nc = _get_compiled()

    scale1 = np.asarray(g1, np.float32) / np.sqrt(np.asarray(v1, np.float32) + EPS)
    shift1 = np.asarray(b1, np.float32) - np.asarray(m1, np.float32) * scale1
    scale2 = np.asarray(g2, np.float32) / np.sqrt(np.asarray(v2, np.float32) + EPS)
    shift2 = np.asarray(b2, np.float32) - np.asarray(m2, np.float32) * scale2

    W1p = (scale1[:, None] * np.asarray(W1, np.float32)).T     # [64, 128] lhsT
    W2p = (scale2[:, None] * np.asarray(W2, np.float32)).T     # [128, 128] lhsT
    lhsT1 = np.concatenate([W1p, -W1p], axis=0)                # [128, 128]
    wstack = np.ascontiguousarray(
        np.concatenate([lhsT1, W2p], axis=1)).astype(np.float16)
    consts = np.ascontiguousarray(np.stack([shift1, shift2], axis=1),
                                  dtype=np.float32)

    in_maps = [
        _prep_core_inputs(b, query_xyz, support_xyz, support_features,
                          neighbor_idx, wstack, consts)
        for b in range(B)
    ]

    res = run_bass_kernel_spmd(nc, in_maps, core_ids=list(range(B)))
    out = np.stack([res.results[b]["out"] for b in range(B)], axis=0)
    kernel.last_results = res
    return out.astype(np.float32)
